# revision 1
# baseline (speedup 1.0000x reference)
"""ArcNegFace loss kernel for 8 TRN2 NeuronCores (model-parallel head).

Sharding: class dim C=100000 over 8 cores (12500/core, padded to 12800).
Per-core inputs (host-prepared, layout/dtype only - no FLOPs on host):
  xn16/xt16: x and x^T cast to fp16; wt: W-shard^T fp16 [512, 12800];
  wfull: full W fp16 (for label-row gather); lidx32/gsc: index metadata.

Device program per core (all under the Tile framework, fp16 compute):
  phase 0: normalize x (Square+accum -> Newton-rsqrt on VectorE, no ScalarE
    table switch); indirect-DMA gather of all 512 label rows from wfull;
    per-row dot -> cos_lb -> arcface target, epilogue bias, scatter values.
  norms pre-pass: stream W^T once; column norms^2 via squares + all-ones
    matmul into PSUM [1, cw]; one batched Newton rsqrt -> rnorm row in DRAM.
    (keeps the main loop free of serial norms chains)
  main loop per 1024-class chunk: re-stream W^T, broadcast rnorm, pre-scale
    (wn = wT*rnorm), fp16 matmuls with a rank-1 all-ones accumulation so
    PSUM holds cos+1 directly, fused epilogue:
      s16 = Square(cosP1/sqrt(2) - (1+target)/sqrt(2))  (ScalarE, (cos-t)^2/2)
      t64 = Exp(-s16 + ln(64*1.2))                      (ScalarE)
      fin = t64*cosP1 - 64                              (VectorE, f32 then fp16)
    fp16 output DMA'd per pair of 128-row chunks.
  label columns fixed by an indirect-DMA scatter of 64*target (per-chunk
  [128,1] offsets - the HW takes one index per partition).
Host: concatenates the 8 [512, 12500] fp16 shards, casts to f32.
"""

import os
import sys

sys.path.insert(0, "/opt/trn_rl_repo")

import numpy as np
import concourse.bacc as bacc
import concourse.mybir as mybir
import concourse.tile as tile
from concourse.bass import IndirectOffsetOnAxis
from concourse.bass_utils import run_bass_kernel_spmd

F32 = mybir.dt.float32
FP16 = mybir.dt.float16
I32 = mybir.dt.int32
Alu = mybir.AluOpType
Act = mybir.ActivationFunctionType

# problem constants
SCALE = 64.0
MARGIN = 0.5
ALPHA = 1.2
SIGMA = 2.0
THRESH = float(np.cos(np.pi - MARGIN))
MM_ = float(np.sin(np.pi - MARGIN) * MARGIN)
COSM = float(np.cos(MARGIN))
SINM = float(np.sin(MARGIN))
SQH = float(1.0 / np.sqrt(SIGMA))          # 1/sqrt(2)
LN64A = float(np.log(SCALE * ALPHA))       # ln(76.8)

N_CORES = 8
B, D, C = 512, 512, 100000
CLOC = C // N_CORES                          # 12500
CPAD = 12800                                 # padded local classes
GSC_SENTINEL = 2**30

MAGIC = 0x5F3759DF
LAST_EXEC_NS = None


def _newton_rsqrt(nc, pool, y, x, tag, n_iter=3):
    """y = 1/sqrt(x) elementwise, pure VectorE (no ScalarE table sets).
    y, x: [P, n] f32 APs (distinct tiles)."""
    P, n = x.shape
    ti = pool.tile([P, n], I32, tag=tag + "_i")
    magic = pool.tile([P, n], I32, tag=tag + "_m")
    xh = pool.tile([P, n], F32, tag=tag + "_h")
    t2 = pool.tile([P, n], F32, tag=tag + "_t")
    nc.vector.memset(magic[:, :], MAGIC)
    nc.vector.tensor_scalar(ti[:, :], x.bitcast(I32), 1, None, Alu.arith_shift_right)
    nc.vector.tensor_tensor(y.bitcast(I32), magic[:, :], ti[:, :], Alu.subtract)
    nc.vector.tensor_scalar(xh[:, :], x, -0.5, None, Alu.mult)
    for _ in range(n_iter):
        nc.vector.tensor_tensor(t2[:, :], y, y, Alu.mult)
        nc.vector.tensor_tensor(t2[:, :], t2[:, :], xh[:, :], Alu.mult)
        nc.vector.tensor_scalar(t2[:, :], t2[:, :], 1.5, None, Alu.add)
        nc.vector.tensor_tensor(y, y, t2[:, :], Alu.mult)


def build(b=B, d=D, cloc=CLOC, cpad=CPAD, n_cores=N_CORES,
          use_scatter=True, nrep=1):
    """Build the per-core SPMD program. All cores run the same graph on their
    own shards."""
    assert b % 128 == 0 and d % 128 == 0 and cpad % 512 == 0
    nb = b // 128           # b chunks (4)
    nd = d // 128           # contraction chunks (4)
    # super-chunk widths over padded classes: multiples of 512, <= 2048
    scw = []
    rem = cpad
    while rem > 0:
        w = min(1024, rem)
        scw.append(w)
        rem -= w

    nc = bacc.Bacc("TRN2", target_bir_lowering=False, debug=False,
                   num_devices=n_cores)

    xn_d = nc.dram_tensor("xn16", [b, d], FP16, kind="ExternalInput")
    xt_d = nc.dram_tensor("xt16", [d, b], FP16, kind="ExternalInput")
    wt_d = nc.dram_tensor("wt", [d, cpad], FP16, kind="ExternalInput")
    cglob = cloc * n_cores
    wf_d = nc.dram_tensor("wfull", [cglob, d], FP16, kind="ExternalInput")
    li_d = nc.dram_tensor("lidx32", [b], I32, kind="ExternalInput")
    gs_d = nc.dram_tensor("gsc", [b], I32, kind="ExternalInput")
    out_d = nc.dram_tensor("out", [b, cloc], FP16, kind="ExternalOutput")

    with tile.TileContext(nc) as tc:
        with (
            tc.tile_pool(name="per", bufs=2) as per,      # persistent tiles
            tc.tile_pool(name="sb", bufs=2) as sb,        # small scratch
            tc.tile_pool(name="wp", bufs=2) as wp,        # w stream tiles
            tc.tile_pool(name="ep", bufs=3) as ep,        # epilogue tiles
            tc.tile_pool(name="ps", bufs=2, space="PSUM") as ps,
            tc.tile_pool(name="ps2", bufs=1, space="PSUM") as ps2,
            tc.tile_pool(name="dr", bufs=2, space="DRAM") as dr,
        ):
            for _rep in range(nrep):
                # ---------------- constants ----------------
                onec = per.tile([128, 1], FP16, tag="onec")       # ones col (lhsT of norm mm)
                one1 = per.tile([1, 128], FP16, tag="one1")       # rank-1 lhsT
                oner = per.tile([1, 512], FP16, tag="oner")       # rank-1 rhs
                bexp = per.tile([128, 1], F32, tag="bexp")        # Exp bias ln(64a)
                nc.gpsimd.memset(onec[:, :], 1.0)
                nc.gpsimd.memset(one1[:, :], 1.0)
                nc.gpsimd.memset(oner[:, :], 1.0)
                nc.gpsimd.memset(bexp[:, :], LN64A)

                # ---------------- phase 0: x ----------------
                xn = [per.tile([128, d], FP16, tag=f"xn{j}", name=f"xn{j}") for j in range(nb)]
                xt = [per.tile([128, b], FP16, tag=f"xt{k}", name=f"xt{k}") for k in range(nd)]
                for j in range(nb):
                    nc.sync.dma_start(xn[j][:, :], xn_d[j * 128:(j + 1) * 128, :])
                for k in range(nd):
                    nc.sync.dma_start(xt[k][:, :], xt_d[k * 128:(k + 1) * 128, :])

                n2x = per.tile([128, nb], F32, tag="n2x")
                xsq = None
                for j in range(nb):
                    xsq = sb.tile([128, d], FP16, tag="xsq")
                    nc.scalar.activation(xsq[:, :], xn[j][:, :], Act.Square,
                                         accum_out=n2x[:, j:j + 1])
                rxn = per.tile([128, nb], F32, tag="rxn")
                _newton_rsqrt(nc, sb, rxn[:, :], n2x[:, :], "nrx")

                # rxnorm -> fp16 -> row [1, b] -> broadcast [128, b] fp16
                rxn16 = sb.tile([128, nb], FP16, tag="rxn16")
                nc.vector.tensor_copy(rxn16[:, :], rxn[:, :])
                xr_dr = dr.tile([b], FP16, tag="xr")
                nc.sync.dma_start(xr_dr[:].rearrange("(j p) -> p j", p=128), rxn16[:, :])
                rxnb = per.tile([128, b], FP16, tag="rxnb")
                nc.sync.dma_start(rxnb[:, :], xr_dr[:].partition_broadcast(128))

                # normalized transposed x (matmul lhsT)
                xnt = [per.tile([128, b], FP16, tag=f"xnt{k}", name=f"xnt{k}") for k in range(nd)]
                for k in range(nd):
                    nc.vector.tensor_tensor(xnt[k][:, :], xt[k][:, :], rxnb[:, :], Alu.mult)

                # ---------------- phase 0: cos at label ----------------
                # every core gathers all 512 label rows from the full W (4 rows
                # per gather element to stay within int16 indices), selects the
                # right row with a per-partition one-hot, and computes cos_lb
                # locally. no cross-core communication.
                gs_sb = per.tile([128, nb], I32, tag="gssb")
                nc.sync.dma_start(gs_sb[:, :], gs_d.ap().rearrange("(j p) -> p j", p=128))
                li_sb = per.tile([128, nb], I32, tag="lisb")
                nc.sync.dma_start(li_sb[:, :], li_d.ap().rearrange("(j p) -> p j", p=128))

                n2g = per.tile([128, nb], F32, tag="n2g")
                rdp = per.tile([128, nb], F32, tag="rdp")
                for j in range(nb):
                    gw = sb.tile([128, d], FP16, tag="gw")
                    nc.gpsimd.indirect_dma_start(
                        gw[:, :], None, wf_d.ap(),
                        IndirectOffsetOnAxis(ap=li_sb[:, j:j + 1], axis=0),
                        bounds_check=cglob - 1, oob_is_err=False)
                    gsq = sb.tile([128, d], FP16, tag="gsq")
                    nc.scalar.activation(gsq[:, :], gw[:, :], Act.Square,
                                         accum_out=n2g[:, j:j + 1])
                    prod = sb.tile([128, d], F32, tag="prod")
                    nc.vector.tensor_tensor(prod[:, :], xn[j][:, :], gw[:, :], Alu.mult)
                    nc.vector.tensor_reduce(rdp[:, j:j + 1], prod[:, :],
                                            mybir.AxisListType.X, Alu.add)
                n2ge = per.tile([128, nb], F32, tag="n2ge")
                nc.vector.tensor_scalar(n2ge[:, :], n2g[:, :], 1e-12, None, Alu.add)
                rgn = per.tile([128, nb], F32, tag="rgn")
                _newton_rsqrt(nc, sb, rgn[:, :], n2ge[:, :], "nrg")
                cosl = per.tile([128, nb], F32, tag="cosl")
                for j in range(nb):
                    nc.vector.tensor_scalar(cosl[:, j:j + 1], rdp[:, j:j + 1],
                                            rxn[:, j:j + 1], rgn[:, j:j + 1],
                                            Alu.mult, Alu.mult)

                # ---------------- phase 0: target ----------------
                c2 = sb.tile([128, nb], F32, tag="tg_c2")
                nc.vector.tensor_tensor(c2[:, :], cosl[:, :], cosl[:, :], Alu.mult)
                v = sb.tile([128, nb], F32, tag="tg_v")
                nc.vector.tensor_scalar(v[:, :], c2[:, :], -1.0, 1.0 + 1e-12,
                                        Alu.mult, Alu.add)
                rs = sb.tile([128, nb], F32, tag="tg_rs")
                _newton_rsqrt(nc, sb, rs[:, :], v[:, :], "nrt")
                sq = sb.tile([128, nb], F32, tag="tg_sq")
                nc.vector.tensor_tensor(sq[:, :], v[:, :], rs[:, :], Alu.mult)
                t1 = sb.tile([128, nb], F32, tag="tg_t1")
                nc.vector.tensor_scalar(t1[:, :], cosl[:, :], COSM, None, Alu.mult)
                t1b = sb.tile([128, nb], F32, tag="tg_t1b")
                nc.vector.tensor_scalar(t1b[:, :], sq[:, :], SINM, None, Alu.mult)
                nc.vector.tensor_tensor(t1[:, :], t1[:, :], t1b[:, :], Alu.subtract)
                t2 = sb.tile([128, nb], F32, tag="tg_t2")
                nc.vector.tensor_scalar(t2[:, :], cosl[:, :], -MM_, None, Alu.add)
                mask = sb.tile([128, nb], F32, tag="tg_mask")
                nc.vector.tensor_scalar(mask[:, :], cosl[:, :], THRESH, None, Alu.is_gt)
                nc.vector.tensor_tensor(t1[:, :], t1[:, :], t2[:, :], Alu.subtract)
                nc.vector.tensor_tensor(t1[:, :], mask[:, :], t1[:, :], Alu.mult)
                tgt = per.tile([128, nb], F32, tag="tgt")
                nc.vector.tensor_tensor(tgt[:, :], t2[:, :], t1[:, :], Alu.add)

                biasc = per.tile([128, nb], F32, tag="biasc")   # -(1+t)/sqrt(2)
                nc.vector.tensor_scalar(biasc[:, :], tgt[:, :], -SQH, -SQH,
                                        Alu.mult, Alu.add)
                val64 = per.tile([128, nb], FP16, tag="val64")   # 64*t
                nc.vector.tensor_scalar(val64[:, :], tgt[:, :], SCALE, None, Alu.mult)

                # ---------------- w column norms pre-pass (streaming) ----------------
                # stream W^T once; column norms^2 -> DRAM; rnorms finalized in
                # 3 staggered batches (separate DRAM tensors -> separate deps)
                # so the main loop starts as soon as batch 0 is ready.
                if cpad >= 8192:
                    bnds = [(0, 2048), (2048, 6144), (6144, cpad)]
                else:
                    bnds = [(0, cpad)]
                nbd_bs = [dr.tile([e - st], FP16, tag=f"nbd{i}", name=f"nbd{i}",
                                  bufs=1) for i, (st, e) in enumerate(bnds)]
                rnd_bs = [dr.tile([e - st], FP16, tag=f"rnd{i}", name=f"rnd{i}",
                                  bufs=1) for i, (st, e) in enumerate(bnds)]

                def _finalize_batch(bi):
                    st, e = bnds[bi]
                    g = (e - st) // 128
                    ncmp = sb.tile([128, g], FP16, tag="ncmp", name="ncmp")
                    nc.sync.dma_start(ncmp[:, :],
                                      nbd_bs[bi][:].rearrange("(p g) -> p g", p=128))
                    nce = sb.tile([128, g], F32, tag="nce", name="nce")
                    nc.vector.tensor_scalar(nce[:, :], ncmp[:, :], 1e-4, None,
                                            Alu.add)
                    rnc = sb.tile([128, g], F32, tag="rnc", name="rnc")
                    _newton_rsqrt(nc, sb, rnc[:, :], nce[:, :], "nrw", n_iter=2)
                    rnc16 = sb.tile([128, g], FP16, tag="rnc16", name="rnc16")
                    nc.vector.tensor_copy(rnc16[:, :], rnc[:, :])
                    nc.sync.dma_start(rnd_bs[bi][:].rearrange("(p g) -> p g", p=128),
                                      rnc16[:, :])

                c0 = 0
                bi = 0
                for sc, cw in enumerate(scw):
                    ncg = cw // 512
                    n2p = ps2.tile([1, cw], mybir.dt.float32, tag="psn", bufs=1,
                                   name="n2p")
                    for k in range(nd):
                        wtn = wp.tile([128, cw], FP16, tag="wtn", bufs=4,
                                      name="wtn")
                        nc.sync.dma_start(wtn[:, :],
                                          wt_d[k * 128:(k + 1) * 128, c0:c0 + cw])
                        sqk = wp.tile([128, cw], FP16, tag="sqk", name="sqk", bufs=3)
                        if k < 2:
                            nc.scalar.activation(sqk[:, :], wtn[:, :], Act.Square)
                        else:
                            nc.vector.tensor_tensor(sqk[:, :], wtn[:, :], wtn[:, :],
                                                    Alu.mult)
                        for m in range(ncg):
                            cs = slice(m * 512, (m + 1) * 512)
                            nc.tensor.matmul(n2p[:, cs], onec[:, :], sqk[:, cs],
                                             start=(k == 0), stop=(k == nd - 1))
                    n2row = sb.tile([1, cw], FP16, tag="n2row")
                    nc.scalar.copy(n2row[:, :], n2p[0:1, :])
                    st, e = bnds[bi]
                    nc.sync.dma_start(nbd_bs[bi][c0 - st:c0 - st + cw], n2row[0:1, :])
                    c0 += cw
                    if c0 >= e:
                        _finalize_batch(bi)
                        bi += 1

                # ---------------- main loop over class super-chunks ----------------
                c0 = 0
                for sc, cw in enumerate(scw):
                    ncg = cw // 512
                    wtall = wp.tile([128, nd * cw], FP16, tag="wtall", bufs=3)
                    for k in range(nd):
                        nc.sync.dma_start(wtall[:, k * cw:(k + 1) * cw],
                                          wt_d[k * 128:(k + 1) * 128, c0:c0 + cw])
                    wt = [wtall[:, k * cw:(k + 1) * cw] for k in range(nd)]
                    rnb = wp.tile([128, cw], FP16, tag="rnb", bufs=3)
                    rbi = next(i for i, (st, e) in enumerate(bnds)
                               if st <= c0 < e)
                    rst = bnds[rbi][0]
                    nc.sync.dma_start(
                        rnb[:, :],
                        rnd_bs[rbi][c0 - rst:c0 - rst + cw].partition_broadcast(128))
                    # normalized W^T columns
                    wn = []
                    for k in range(nd):
                        wnk = wp.tile([128, cw], FP16, tag=f"wn{k}", bufs=3)
                        nc.vector.tensor_tensor(wnk[:, :], wt[k], rnb[:, :], Alu.mult)
                        wn.append(wnk)
                    # matmul + epilogue per 128-row b chunk
                    wr = min(cw, cloc - c0)   # real (non-pad) columns to write
                    finp = [ep.tile([128, 2 * cw], FP16, tag="finp0", name="finp0",
                                    bufs=3),
                            ep.tile([128, 2 * cw], FP16, tag="finp1", name="finp1",
                                    bufs=3)]
                    for j in range(nb):
                        acc = ps.tile([128, cw], mybir.dt.float32, tag="ps",
                                      name="acc", bufs=3)
                        for m in range(ncg):
                            cs = slice(m * 512, (m + 1) * 512)
                            for k in range(nd):
                                nc.tensor.matmul(acc[:, cs],
                                                 xnt[k][:, j * 128:(j + 1) * 128],
                                                 wn[k][:, cs],
                                                 start=(k == 0), stop=False)
                            nc.tensor.matmul(acc[:, cs], one1[:, :], oner[:, :],
                                             start=False, stop=True)
                        s16 = ep.tile([128, cw], FP16, tag="s16", bufs=2)
                        nc.scalar.activation(s16[:, :], acc[:, :], Act.Square,
                                             bias=biasc[:, j:j + 1], scale=SQH)
                        t64 = ep.tile([128, cw], F32, tag="t64", bufs=2)
                        nc.scalar.activation(t64[:, :], s16[:, :], Act.Exp,
                                             bias=bexp[:, 0:1], scale=-1.0)
                        fsl = finp[j // 2][:, (j % 2) * cw:(j % 2) * cw + cw]
                        pf = ep.tile([128, cw], F32, tag="pf", bufs=2)
                        nc.vector.tensor_tensor(pf[:, :], t64[:, :], acc[:, :], Alu.mult)
                        nc.vector.tensor_scalar(fsl, pf[:, :], -SCALE, None, Alu.add)
                        if wr > 0 and j % 2 == 1:
                            g = j // 2
                            nc.sync.dma_start(
                                out_d[g * 256:(g + 1) * 256, c0:c0 + wr].rearrange(
                                    "(j p) c -> p j c", p=128),
                                finp[g].rearrange("p (j c) -> p j c", j=2)[:, :, :wr])
                    c0 += cw

                # ---------------- label-column fixup scatter ----------------
                # one scatter per 128-row chunk: HW indirect DMA takes one index
                # per partition ([128, 1] offsets), matching the gathers above.
                out_flat = out_d.ap().rearrange("b c -> (b c)").rearrange(
                    "(o x) -> o x", o=1)
                for j in range(nb if use_scatter else 0):
                    nc.gpsimd.indirect_dma_start(
                        out_flat,
                        IndirectOffsetOnAxis(ap=gs_sb[:, j:j + 1], axis=1),
                        val64[:, j:j + 1], None,
                        bounds_check=b * cloc - 1, oob_is_err=False)

    nc.compile()
    return nc


def _prep_in_maps(x, label, weight, cloc=CLOC, cpad=CPAD, n_cores=N_CORES):
    b = x.shape[0]
    x32 = np.ascontiguousarray(np.asarray(x, dtype=np.float32))
    w32 = np.ascontiguousarray(np.asarray(weight, dtype=np.float32))
    lab = np.asarray(label).astype(np.int64)
    xn16 = x32.astype(np.float16)
    xt16 = np.ascontiguousarray(x32.T).astype(np.float16)
    cglob = cloc * n_cores
    wfull = w32[:cglob].astype(np.float16)
    lidx32 = lab.astype(np.int32)
    in_maps = []
    for i in range(n_cores):
        lo = i * cloc
        wsh = w32[lo:lo + cloc]
        wt = np.zeros((wsh.shape[1], cpad), np.float16)
        wt[:, :cloc] = wsh.T.astype(np.float16)
        valid = (lab >= lo) & (lab < lo + cloc)
        gsc = np.where(valid, np.arange(b, dtype=np.int64) * cloc + (lab - lo),
                       GSC_SENTINEL).astype(np.int32)
        in_maps.append({
            "xn16": xn16,
            "xt16": xt16,
            "wt": wt,
            "wfull": wfull,
            "lidx32": lidx32,
            "gsc": gsc,
        })
    return in_maps


_BUILD_CACHE = {}


def kernel(input, label, weight):
    """Full inputs in, full [512, 100000] f32 logits out."""
    global LAST_EXEC_NS
    key = "full"
    if key not in _BUILD_CACHE:
        _BUILD_CACHE[key] = build()
    nc = _BUILD_CACHE[key]
    in_maps = _prep_in_maps(input, label, weight)
    trace = bool(int(os.environ.get("KBENCH_TRACE", "0")))
    res = run_bass_kernel_spmd(nc, in_maps, core_ids=list(range(N_CORES)),
                               trace=trace)
    LAST_EXEC_NS = res.exec_time_ns
    out = np.concatenate([np.asarray(r["out"]) for r in res.results], axis=1)
    return np.ascontiguousarray(out.astype(np.float32))



# revision 17
# speedup vs baseline: 7.6190x; 7.6190x over previous
"""ArcNegFace loss kernel for 8 TRN2 NeuronCores (model-parallel head).

Sharding: class dim C=100000 over 8 cores (12500/core, padded to 12800).
Per-core inputs (host-prepared, layout/dtype only - no FLOPs on host):
  xn16/xt16: x and x^T cast to fp16; wt: W-shard^T fp16 [512, 12800];
  wfull: full W fp16 (for label-row gather); lidx32/gsc: index metadata.

v2 layout (single W pass, W resident in SBUF):
  phase 0: normalize x (Newton-rsqrt on VectorE); indirect-DMA gather of
    all 512 label rows from wfull; per-row dot -> cos_lb -> arcface target.
  W streamed ONCE into 4 persistent SBUF tiles [128, 12800] (52 chunk DMAs);
  column norms^2 per 1024-chunk: squares on GpSimdE (frees ACT/DVE),
  ones-matmul into PSUM [1, cw]; finalized in 3 staggered batches
  (DRAM reshape hop -> batched Newton rsqrt -> rnorm row) so the main loop
  starts after ~2 chunks of W have arrived.
  rnorm broadcast once per batch into a persistent [128, 12800] fp16 tile.
  main loop per chunk: wn = wt*rnorm (VectorE fp16 2x), fp16 matmuls with a
  rank-1 all-ones accumulation so PSUM holds cos+1, fused epilogue:
      s16 = Square(cosP1/sqrt(2) - (1+target)/sqrt(2))  (ScalarE)
      t64 = Exp(-s16 + ln(64*1.2))                      (ScalarE)
      pf  = t64*cosP1 (VectorE); fin = pf - 64 -> fp16  (VectorE)
  label-column fixup: 4 indirect-DMA scatters with PER-PARTITION column
  offsets into out.rearrange("(j p) c -> p (j c)") - the fast 128-descriptor
  SWDGE path (the old flat [1, B*C] view generated per-element descriptors).
Host: concatenates the 8 [512, 12500] fp16 shards, casts to f32.
"""

import os
import sys

sys.path.insert(0, "/opt/trn_rl_repo")

import numpy as np
import concourse.bacc as bacc
import concourse.mybir as mybir
import concourse.tile as tile
from concourse.bass import IndirectOffsetOnAxis
from concourse.bass_utils import run_bass_kernel_spmd

F32 = mybir.dt.float32
FP16 = mybir.dt.float16
I32 = mybir.dt.int32
Alu = mybir.AluOpType
Act = mybir.ActivationFunctionType

# problem constants
SCALE = 64.0
MARGIN = 0.5
ALPHA = 1.2
SIGMA = 2.0
THRESH = float(np.cos(np.pi - MARGIN))
MM_ = float(np.sin(np.pi - MARGIN) * MARGIN)
COSM = float(np.cos(MARGIN))
SINM = float(np.sin(MARGIN))
SQH = float(1.0 / np.sqrt(SIGMA))          # 1/sqrt(2)
LN64A = float(np.log(SCALE * ALPHA))       # ln(76.8)

N_CORES = 8
B, D, C = 512, 512, 100000
CLOC = C // N_CORES                          # 12500
CPAD = 12800                                 # padded local classes
GSC_SENTINEL = 2**30

MAGIC = 0x5F3759DF
LAST_EXEC_NS = None


def _newton_rsqrt(nc, pool, y, x, tag, n_iter=3):
    """y = 1/sqrt(x) elementwise, pure VectorE (no ScalarE table sets).
    y, x: [P, n] f32 APs (distinct tiles)."""
    P, n = x.shape
    ti = pool.tile([P, n], I32, tag=tag + "_i")
    magic = pool.tile([P, n], I32, tag=tag + "_m")
    xh = pool.tile([P, n], F32, tag=tag + "_h")
    t2 = pool.tile([P, n], F32, tag=tag + "_t")
    nc.vector.memset(magic[:, :], MAGIC)
    nc.vector.tensor_scalar(ti[:, :], x.bitcast(I32), 1, None, Alu.arith_shift_right)
    nc.vector.tensor_tensor(y.bitcast(I32), magic[:, :], ti[:, :], Alu.subtract)
    nc.vector.tensor_scalar(xh[:, :], x, -0.5, None, Alu.mult)
    for _ in range(n_iter):
        nc.vector.tensor_tensor(t2[:, :], y, y, Alu.mult)
        nc.vector.tensor_tensor(t2[:, :], t2[:, :], xh[:, :], Alu.mult)
        nc.vector.tensor_scalar(t2[:, :], t2[:, :], 1.5, None, Alu.add)
        nc.vector.tensor_tensor(y, y, t2[:, :], Alu.mult)


def build(b=B, d=D, cloc=CLOC, cpad=CPAD, n_cores=N_CORES,
          use_scatter=True, nrep=1):
    """Build the per-core SPMD program. All cores run the same graph on their
    own shards."""
    assert b % 128 == 0 and d % 128 == 0 and cpad % 512 == 0
    nb = b // 128           # b chunks (4)
    nd = d // 128           # contraction chunks (4)
    # chunk widths over padded classes
    scw = []
    rem = cpad
    while rem > 0:
        w = min(1024, rem)
        scw.append(w)
        rem -= w
    csum = np.cumsum([0] + scw)              # chunk start cols
    nch = len(scw)
    # norm-finalize batches (by chunk index): small first batch so the main
    # loop can start early; widths must be multiples of 128
    if nch >= 10:
        bat_ch = [(0, 1), (1, 3), (3, 7), (7, nch)]
    elif nch >= 6:
        bat_ch = [(0, 2), (2, 5), (5, nch)]
    else:
        bat_ch = [(0, nch)]
    bnds = [(int(csum[s]), int(csum[e])) for s, e in bat_ch]

    nc = bacc.Bacc("TRN2", target_bir_lowering=False, debug=False,
                   num_devices=n_cores)

    xn_d = nc.dram_tensor("xn16", [b, d], FP16, kind="ExternalInput")
    xt_d = nc.dram_tensor("xt16", [d, b], FP16, kind="ExternalInput")
    wt_d = nc.dram_tensor("wt", [d, cpad], FP16, kind="ExternalInput")
    cglob = cloc * n_cores
    wf_d = nc.dram_tensor("wfull", [cglob, d], FP16, kind="ExternalInput")
    li_d = nc.dram_tensor("lidx32", [b], I32, kind="ExternalInput")
    gs_d = nc.dram_tensor("gsc", [b], I32, kind="ExternalInput")
    out_d = nc.dram_tensor("out", [b, cloc], FP16, kind="ExternalOutput")

    with tile.TileContext(nc) as tc:
        with (
            tc.tile_pool(name="per", bufs=1) as per,      # persistent tiles
            tc.tile_pool(name="sb", bufs=2) as sb,        # small scratch
            tc.tile_pool(name="wp", bufs=2) as wp,        # wn/sq stream tiles
            tc.tile_pool(name="ep", bufs=2) as ep,        # epilogue tiles
            tc.tile_pool(name="ps", bufs=3, space="PSUM") as ps,
            tc.tile_pool(name="ps2", bufs=1, space="PSUM") as ps2,
            tc.tile_pool(name="dr", bufs=1, space="DRAM") as dr,
        ):
            for _rep in range(nrep):
                # ---------------- constants ----------------
                onec = per.tile([128, 1], FP16, tag="onec")       # ones col (lhsT of norm mm)
                one1 = per.tile([1, 128], FP16, tag="one1")       # rank-1 lhsT
                oner = per.tile([1, 512], FP16, tag="oner")       # rank-1 rhs
                bexp = per.tile([128, 1], F32, tag="bexp")        # Exp bias ln(64a)
                nc.gpsimd.memset(onec[:, :], 1.0)
                nc.gpsimd.memset(one1[:, :], 1.0)
                nc.gpsimd.memset(oner[:, :], 1.0)
                nc.gpsimd.memset(bexp[:, :], LN64A)

                # ---------------- input DMAs ----------------
                xn = [per.tile([128, d], FP16, tag=f"xn{j}", name=f"xn{j}") for j in range(nb)]
                xt = [per.tile([128, b], FP16, tag=f"xt{k}", name=f"xt{k}") for k in range(nd)]
                for j in range(nb):
                    nc.sync.dma_start(xn[j][:, :], xn_d[j * 128:(j + 1) * 128, :])
                for k in range(nd):
                    nc.sync.dma_start(xt[k][:, :], xt_d[k * 128:(k + 1) * 128, :])
                gs_sb = per.tile([128, nb], I32, tag="gssb")
                nc.sync.dma_start(gs_sb[:, :], gs_d.ap().rearrange("(j p) -> p j", p=128))
                li_sb = per.tile([128, nb], I32, tag="lisb")
                nc.sync.dma_start(li_sb[:, :], li_d.ap().rearrange("(j p) -> p j", p=128))

                # W resident in SBUF; per-(chunk, k) DMAs are issued inside
                # _norm_chunk so small finalize DMAs aren't queued behind the
                # whole W stream on the in-order sync engine
                wtk = [per.tile([128, cpad], FP16, tag=f"wtk{k}", name=f"wtk{k}") for k in range(nd)]

                # ---------------- phase 0: x ----------------
                n2x = per.tile([128, nb], F32, tag="n2x")
                for j in range(nb):
                    xsq = sb.tile([128, d], FP16, tag="xsq")
                    nc.scalar.activation(xsq[:, :], xn[j][:, :], Act.Square,
                                         accum_out=n2x[:, j:j + 1])
                rxn = per.tile([128, nb], F32, tag="rxn")
                _newton_rsqrt(nc, sb, rxn[:, :], n2x[:, :], "nrx")

                # rxnorm -> fp16 -> row [1, b] -> broadcast [128, b] fp16
                rxn16 = sb.tile([128, nb], FP16, tag="rxn16")
                nc.vector.tensor_copy(rxn16[:, :], rxn[:, :])
                xr_dr = dr.tile([b], FP16, tag="xr")
                nc.sync.dma_start(xr_dr[:].rearrange("(j p) -> p j", p=128), rxn16[:, :])
                rxnb = per.tile([128, b], FP16, tag="rxnb")
                nc.sync.dma_start(rxnb[:, :], xr_dr[:].partition_broadcast(128))

                # normalized transposed x (matmul lhsT)
                xnt = [per.tile([128, b], FP16, tag=f"xnt{k}", name=f"xnt{k}") for k in range(nd)]
                for k in range(nd):
                    nc.vector.tensor_tensor(xnt[k][:, :], xt[k][:, :], rxnb[:, :], Alu.mult)

                # ---------------- phase 0: cos at label ----------------
                # every core gathers all 512 label rows from the full W and
                # computes cos_lb locally. no cross-core communication.
                n2g = per.tile([128, nb], F32, tag="n2g")
                rdp = per.tile([128, nb], F32, tag="rdp")
                for j in range(nb):
                    gw = sb.tile([128, d], FP16, tag="gw")
                    nc.gpsimd.indirect_dma_start(
                        gw[:, :], None, wf_d.ap(),
                        IndirectOffsetOnAxis(ap=li_sb[:, j:j + 1], axis=0),
                        bounds_check=cglob - 1, oob_is_err=False)
                    gsq = sb.tile([128, d], FP16, tag="gsq")
                    nc.scalar.activation(gsq[:, :], gw[:, :], Act.Square,
                                         accum_out=n2g[:, j:j + 1])
                    prod = sb.tile([128, d], F32, tag="prod")
                    nc.vector.tensor_tensor(prod[:, :], xn[j][:, :], gw[:, :], Alu.mult)
                    nc.vector.tensor_reduce(rdp[:, j:j + 1], prod[:, :],
                                            mybir.AxisListType.X, Alu.add)
                n2ge = per.tile([128, nb], F32, tag="n2ge")
                nc.vector.tensor_scalar(n2ge[:, :], n2g[:, :], 1e-12, None, Alu.add)
                rgn = per.tile([128, nb], F32, tag="rgn")
                _newton_rsqrt(nc, sb, rgn[:, :], n2ge[:, :], "nrg")
                cosl = per.tile([128, nb], F32, tag="cosl")
                for j in range(nb):
                    nc.vector.tensor_scalar(cosl[:, j:j + 1], rdp[:, j:j + 1],
                                            rxn[:, j:j + 1], rgn[:, j:j + 1],
                                            Alu.mult, Alu.mult)

                # ---------------- phase 0: target ----------------
                c2 = sb.tile([128, nb], F32, tag="tg_c2")
                nc.vector.tensor_tensor(c2[:, :], cosl[:, :], cosl[:, :], Alu.mult)
                v = sb.tile([128, nb], F32, tag="tg_v")
                nc.vector.tensor_scalar(v[:, :], c2[:, :], -1.0, 1.0 + 1e-12,
                                        Alu.mult, Alu.add)
                rs = sb.tile([128, nb], F32, tag="tg_rs")
                _newton_rsqrt(nc, sb, rs[:, :], v[:, :], "nrt")
                sq = sb.tile([128, nb], F32, tag="tg_sq")
                nc.vector.tensor_tensor(sq[:, :], v[:, :], rs[:, :], Alu.mult)
                t1 = sb.tile([128, nb], F32, tag="tg_t1")
                nc.vector.tensor_scalar(t1[:, :], cosl[:, :], COSM, None, Alu.mult)
                t1b = sb.tile([128, nb], F32, tag="tg_t1b")
                nc.vector.tensor_scalar(t1b[:, :], sq[:, :], SINM, None, Alu.mult)
                nc.vector.tensor_tensor(t1[:, :], t1[:, :], t1b[:, :], Alu.subtract)
                t2 = sb.tile([128, nb], F32, tag="tg_t2")
                nc.vector.tensor_scalar(t2[:, :], cosl[:, :], -MM_, None, Alu.add)
                mask = sb.tile([128, nb], F32, tag="tg_mask")
                nc.vector.tensor_scalar(mask[:, :], cosl[:, :], THRESH, None, Alu.is_gt)
                nc.vector.tensor_tensor(t1[:, :], t1[:, :], t2[:, :], Alu.subtract)
                nc.vector.tensor_tensor(t1[:, :], mask[:, :], t1[:, :], Alu.mult)
                tgt = per.tile([128, nb], F32, tag="tgt")
                nc.vector.tensor_tensor(tgt[:, :], t2[:, :], t1[:, :], Alu.add)

                biasc = per.tile([128, nb], F32, tag="biasc")   # -(1+t)/sqrt(2)
                nc.vector.tensor_scalar(biasc[:, :], tgt[:, :], -SQH, -SQH,
                                        Alu.mult, Alu.add)
                probz = None
                if _rep > 0:
                    # timing builds only (nrep>1): chain reps through a
                    # zero-scaled probe of the previous rep's output so reps
                    # are neither dead-code-eliminated nor overlapped -
                    # per-rep delta then measures single-run latency
                    probe = sb.tile([128, nb], FP16, tag="probe")
                    nc.sync.dma_start(probe[:, :],
                                      out_d[0:128, cloc - nb:cloc])
                    probz = sb.tile([128, nb], F32, tag="probz")
                    nc.vector.tensor_scalar(probz[:, :], probe[:, :], 0.0, None,
                                            Alu.mult)
                    nc.vector.tensor_tensor(biasc[:, :], biasc[:, :],
                                            probz[:, :], Alu.add)
                val64 = per.tile([128, nb], FP16, tag="val64")   # 64*t
                nc.vector.tensor_scalar(val64[:, :], tgt[:, :], SCALE, None, Alu.mult)

                # ---------------- w column norms (streaming, batched) ----------
                nbd_bs = [dr.tile([e - st], FP16, tag=f"nbd{i}", name=f"nbd{i}")
                          for i, (st, e) in enumerate(bnds)]
                rnd_bs = [dr.tile([e - st], FP16, tag=f"rnd{i}", name=f"rnd{i}")
                          for i, (st, e) in enumerate(bnds)]

                def _w_batch(bi):
                    for c in range(*bat_ch[bi]):
                        c0, cw = int(csum[c]), scw[c]
                        for k in range(nd):
                            nc.sync.dma_start(
                                wtk[k][:, c0:c0 + cw],
                                wt_d[k * 128:(k + 1) * 128, c0:c0 + cw])

                def _norm_chunk(c):
                    c0, cw = int(csum[c]), scw[c]
                    ncg = cw // 512
                    n2p = ps2.tile([1, 1024], mybir.dt.float32, tag="psn")
                    for k in range(nd):
                        sqk = wp.tile([128, 1024], FP16, tag="sqk", bufs=3)
                        nc.gpsimd.tensor_tensor(sqk[:, :cw], wtk[k][:, c0:c0 + cw],
                                                wtk[k][:, c0:c0 + cw], Alu.mult)
                        for m in range(ncg):
                            cs = slice(m * 512, (m + 1) * 512)
                            nc.tensor.matmul(n2p[:, cs], onec[:, :], sqk[:, cs],
                                             start=(k == 0), stop=(k == nd - 1))
                    n2row = sb.tile([1, 1024], FP16, tag="n2row")
                    nc.scalar.copy(n2row[:, :cw], n2p[0:1, :cw])
                    bi = next(i for i, (s, e) in enumerate(bat_ch) if s <= c < e)
                    st = bnds[bi][0]
                    nc.sync.dma_start(nbd_bs[bi][c0 - st:c0 - st + cw],
                                      n2row[0:1, :cw])

                def _finalize_batch(bi):
                    st, e = bnds[bi]
                    g = (e - st) // 128
                    ncmp = sb.tile([128, g], FP16, tag="ncmp")
                    nc.sync.dma_start(ncmp[:, :],
                                      nbd_bs[bi][:].rearrange("(p g) -> p g", p=128))
                    nce = sb.tile([128, g], F32, tag="nce")
                    if probz is None:
                        nc.vector.tensor_scalar(nce[:, :], ncmp[:, :], 1e-4,
                                                None, Alu.add)
                    else:
                        # rep-chaining gate: probz == 0, so numerics unchanged
                        nc.vector.tensor_scalar(nce[:, :], ncmp[:, :],
                                                probz[:, 0:1], 1e-4,
                                                Alu.add, Alu.add)
                    rnc = sb.tile([128, g], F32, tag="rnc")
                    _newton_rsqrt(nc, sb, rnc[:, :], nce[:, :], "nrw", n_iter=2)
                    rnc16 = sb.tile([128, g], FP16, tag="rnc16")
                    nc.vector.tensor_copy(rnc16[:, :], rnc[:, :])
                    nc.sync.dma_start(rnd_bs[bi][:].rearrange("(p g) -> p g", p=128),
                                      rnc16[:, :])

                def _main_chunk(c):
                    c0, cw = int(csum[c]), scw[c]
                    ncg = cw // 512
                    bi = next(i for i, (s, e) in enumerate(bat_ch) if s <= c < e)
                    st = bnds[bi][0]
                    rnb = wp.tile([128, 1024], FP16, tag="rnb", bufs=3)
                    nc.sync.dma_start(
                        rnb[:, :cw],
                        rnd_bs[bi][c0 - st:c0 - st + cw].partition_broadcast(128))
                    wn = []
                    for k in range(nd):
                        wnk = wp.tile([128, 1024], FP16, tag=f"wn{k}", bufs=2)
                        nc.vector.tensor_tensor(wnk[:, :cw], wtk[k][:, c0:c0 + cw],
                                                rnb[:, :cw], Alu.mult)
                        wn.append(wnk)
                    wr = min(cw, cloc - c0)   # real (non-pad) columns to write
                    finp = [ep.tile([128, 2 * 1024], FP16, tag=f"finp{h}", name=f"finp{h}", bufs=3)
                            for h in range(2)]
                    for j in range(nb):
                        acc = ps.tile([128, 1024], mybir.dt.float32, tag="acc",
                                      bufs=3)
                        for m in range(ncg):
                            cs = slice(m * 512, (m + 1) * 512)
                            for k in range(nd):
                                nc.tensor.matmul(acc[:, cs],
                                                 xnt[k][:, j * 128:(j + 1) * 128],
                                                 wn[k][:, cs],
                                                 start=(k == 0), stop=False)
                            nc.tensor.matmul(acc[:, cs], one1[:, :], oner[:, :],
                                             start=False, stop=True)
                        s16 = ep.tile([128, 1024], FP16, tag="s16", bufs=2)
                        nc.scalar.activation(s16[:, :cw], acc[:, :cw], Act.Square,
                                             bias=biasc[:, j:j + 1], scale=SQH)
                        t64 = ep.tile([128, 1024], F32, tag="t64", bufs=2)
                        nc.scalar.activation(t64[:, :cw], s16[:, :cw], Act.Exp,
                                             bias=bexp[:, 0:1], scale=-1.0)
                        pf = ep.tile([128, 1024], F32, tag="pf", bufs=2)
                        nc.vector.tensor_tensor(pf[:, :cw], t64[:, :cw],
                                                acc[:, :cw], Alu.mult)
                        fsl = finp[j // 2][:, (j % 2) * cw:(j % 2) * cw + cw]
                        nc.vector.tensor_scalar(fsl, pf[:, :cw], -SCALE, None,
                                                Alu.add)
                        if wr > 0 and j % 2 == 1:
                            g = j // 2
                            nc.sync.dma_start(
                                out_d[g * 256:(g + 1) * 256, c0:c0 + wr].rearrange(
                                    "(j p) c -> p j c", p=128),
                                finp[g][:, :2 * cw].rearrange(
                                    "p (j c) -> p j c", j=2)[:, :, :wr])

                # interleave: batch bi's W DMAs are issued before batch bi-1's
                # main loop (they stream while PE is busy); batch bi's norms +
                # finalize are programmed after the FIRST main chunk of batch
                # bi-1 so PE isn't stalled waiting for the W arrival, but the
                # finalize chain still completes well before main(bi) starts
                _w_batch(0)
                for c in range(*bat_ch[0]):
                    _norm_chunk(c)
                _finalize_batch(0)
                for bi in range(1, len(bat_ch)):
                    _w_batch(bi)
                    s, e = bat_ch[bi - 1]
                    _main_chunk(s)
                    for c in range(*bat_ch[bi]):
                        _norm_chunk(c)
                    _finalize_batch(bi)
                    for c in range(s + 1, e):
                        _main_chunk(c)
                s, e = bat_ch[-1]
                for c in range(s, e):
                    _main_chunk(c)

                # ---------------- label-column fixup scatter ----------------
                # flat per-partition offsets (r*cloc + col); sentinel = OOB
                out_flat = out_d.ap().rearrange("b c -> (b c)").rearrange(
                    "(o x) -> o x", o=1)
                for j in range(nb if use_scatter else 0):
                    nc.gpsimd.indirect_dma_start(
                        out_flat,
                        IndirectOffsetOnAxis(ap=gs_sb[:, j:j + 1], axis=1),
                        val64[:, j:j + 1], None,
                        bounds_check=b * cloc - 1, oob_is_err=False)
                if not use_scatter:
                    # host fixup path: export 64*t per row instead
                    tv_d = nc.dram_tensor("tval", [b], FP16,
                                          kind="ExternalOutput") if _rep == 0 \
                        else tv_d
                    nc.sync.dma_start(
                        tv_d.ap().rearrange("(j p) -> p j", p=128), val64[:, :])

    nc.compile()
    return nc


def _prep_in_maps(x, label, weight, cloc=CLOC, cpad=CPAD, n_cores=N_CORES):
    b = x.shape[0]
    x32 = np.ascontiguousarray(np.asarray(x, dtype=np.float32))
    w32 = np.ascontiguousarray(np.asarray(weight, dtype=np.float32))
    lab = np.asarray(label).astype(np.int64)
    xn16 = x32.astype(np.float16)
    xt16 = np.ascontiguousarray(x32.T).astype(np.float16)
    cglob = cloc * n_cores
    wfull = w32[:cglob].astype(np.float16)
    lidx32 = lab.astype(np.int32)
    in_maps = []
    for i in range(n_cores):
        lo = i * cloc
        wsh = w32[lo:lo + cloc]
        wt = np.zeros((wsh.shape[1], cpad), np.float16)
        wt[:, :cloc] = wsh.T.astype(np.float16)
        valid = (lab >= lo) & (lab < lo + cloc)
        gsc = np.where(valid, np.arange(b, dtype=np.int64) * cloc + (lab - lo),
                       GSC_SENTINEL).astype(np.int32)
        in_maps.append({
            "xn16": xn16,
            "xt16": xt16,
            "wt": wt,
            "wfull": wfull,
            "lidx32": lidx32,
            "gsc": gsc,
        })
    return in_maps


_BUILD_CACHE = {}


def kernel(input, label, weight):
    """Full inputs in, full [512, 100000] f32 logits out."""
    global LAST_EXEC_NS
    key = "full"
    if key not in _BUILD_CACHE:
        _BUILD_CACHE[key] = build()
    nc = _BUILD_CACHE[key]
    in_maps = _prep_in_maps(input, label, weight)
    trace = bool(int(os.environ.get("KBENCH_TRACE", "0")))
    res = run_bass_kernel_spmd(nc, in_maps, core_ids=list(range(N_CORES)),
                               trace=trace)
    LAST_EXEC_NS = res.exec_time_ns
    out = np.concatenate([np.asarray(r["out"]) for r in res.results], axis=1)
    out = np.ascontiguousarray(out.astype(np.float32))
    if "tval" in res.results[0]:
        # host fixup path: place 64*t at the label columns (indexing only)
        lab = np.asarray(label).astype(np.int64)
        tv = np.asarray(res.results[0]["tval"]).astype(np.float32)
        out[np.arange(out.shape[0]), lab] = tv
    return out
